# revision 1
# baseline (speedup 1.0000x reference)
"""Trainium2 Bass kernel for DiagonalColCausalLinear.

Computes out[b,e,t] = sum_{s<t} x[b,e,s] * v[s] * d^(t-s) + x[b,e,t] * v2[t] + bias[t]
with d = clip(decay_value[1,0], 0.9, 1.0), v = weight, v2 = diag_weight.

Sharding: data-parallel over batch B across the 8 cores; the small parameter
tensors are replicated. x is pre-transposed to (S, E) on the host (a pure
layout change, folded into the shard/distribute step) so the device reads the
sequence axis on partitions directly -- no on-device transposes needed.

Device algorithm (per core; x^T in DRAM as (S, E)):
  Chunked causal scan along the sequence axis (chunk C=128), O(E*S*C) work
  instead of the O(E*S^2) dense matmul:
    - within-chunk triangular matmuls: psum[e, t] += xT_c^T @ T''_c where
      T''_c[s_l, t_l] = v[s]*d^(t_l-s_l) above the diagonal, v2[s] on it
    - cross-chunk carries via accumulating matmuls vs R:
      carry[e, c'] = sum_{s < c'*C} x[e,s]*v[s]*d^(c'*C - s)
    - carry applied as a per-partition bias fused into the PSUM->SBUF
      copy-out (ScalarE activation bias / DVE tensor_scalar, split across
      both engines), or via a rank-1 PE matmul against d^{t_l} when d != 1
  All matmuls run in fp32 (full precision).
  e is processed in 4 column groups so compute starts after 1/4 of the
  input has streamed in; outputs ship in halves on the ScalarE HWDGE ring,
  keeping a separate DMA FIFO from the input loads on the sync ring.
"""
import numpy as np

import concourse.bass as bass
import concourse.mybir as mybir
import concourse.tile as tile
import concourse.bacc as bacc
from concourse import bass_utils

F32 = mybir.dt.float32

B, E, S = 8, 2048, 2048
N_CORES = 8
PT = 128            # partition tile
C = 128             # scan chunk == one k-subchunk
NCH = S // C        # 16 chunks
NE = E // PT        # 16 e-tiles per core
NSC = S // PT       # 16 s-subchunks
# e-tiles per DMA/compute pipeline group (compute for a group starts once
# its slice of the input stream has landed)
EGROUPS = [4, 4, 4, 4]
G = len(EGROUPS)

_prog_cache: dict = {}


def _build_constants(v: np.ndarray, v2: np.ndarray, d: float):
    """Host-side (tiny, O(S*C)) constant matrices encoding the decay structure."""
    Tm = np.zeros((NSC, PT, C), np.float32)
    Rm = np.zeros((NSC, PT, NCH), np.float32)
    t_local = np.arange(C)
    cc = np.arange(NCH)
    for sc in range(NSC):
        s_in_chunk = np.arange(PT)
        s_glob = sc * C + s_in_chunk
        diff = t_local[None, :] - s_in_chunk[:, None]
        with np.errstate(over="ignore", invalid="ignore"):
            Tm[sc] = np.where(diff > 0, v[s_glob][:, None] * (d ** np.maximum(diff, 0)), 0.0)
        Tm[sc][np.arange(PT), s_in_chunk] = v2[s_glob]
        # R[a, c'] = v[s] * d^(c'*C - s) for chunks c' > sc (carry to chunk start)
        expo = cc[None, :] * C - s_glob[:, None]
        with np.errstate(over="ignore", invalid="ignore"):
            Rm[sc] = np.where(cc[None, :] > sc, v[s_glob][:, None] * (d ** np.maximum(expo, 0)), 0.0)
    dpow = (d ** t_local).astype(np.float32).reshape(1, C)
    return Tm, Rm, dpow


def _build_program(d_is_one: bool, repeats: int = 1, loop_n: int | None = None):
    key = (d_is_one, repeats, loop_n)
    if key in _prog_cache:
        return _prog_cache[key]

    nc = bacc.Bacc("TRN2", target_bir_lowering=False, debug=False, num_devices=1)
    xt_d = nc.dram_tensor("xt", [S, E], F32, kind="ExternalInput").ap()
    tmat_d = nc.dram_tensor("tmat", [PT, NSC * C], F32, kind="ExternalInput").ap()
    rmat_d = nc.dram_tensor("rmat", [PT, NSC * NCH], F32, kind="ExternalInput").ap()
    dpow_d = nc.dram_tensor("dpow", [1, C], F32, kind="ExternalInput").ap()
    ident_d = nc.dram_tensor("ident", [PT, PT], F32, kind="ExternalInput").ap()
    out_d = nc.dram_tensor("out", [E, S], F32, kind="ExternalOutput").ap()

    with tile.TileContext(nc) as tc:
        with (
            tc.tile_pool(name="const", bufs=1) as cpool,
            tc.tile_pool(name="xt", bufs=NSC) as xtpool,
            tc.tile_pool(name="outp", bufs=6) as opool,
            tc.tile_pool(name="small", bufs=4) as spool,
            tc.tile_pool(name="psm", bufs=6, space="PSUM") as psm,
            tc.tile_pool(name="pscy", bufs=2, space="PSUM") as pscy,
        ):
            # constants (resident) -- loaded via SWDGE so they don't sit
            # ahead of the x stream in the SP HWDGE FIFO
            tmat = cpool.tile([PT, NSC * C], F32, tag="tmat")
            nc.gpsimd.dma_start(tmat[:, :], tmat_d[:, :])
            rmat = cpool.tile([PT, NSC * NCH], F32, tag="rmat")
            nc.gpsimd.dma_start(rmat[:, :], rmat_d[:, :])
            if not d_is_one:
                dpow = cpool.tile([1, C], F32, tag="dpow")
                nc.gpsimd.dma_start(dpow[:, :], dpow_d[:, :])
                ident = cpool.tile([PT, PT], F32, tag="ident")
                nc.gpsimd.dma_start(ident[:, :], ident_d[:, :])

            import contextlib
            loop_cm = tc.For_i(0, loop_n, 1) if loop_n else contextlib.nullcontext()
            with loop_cm:
              for _rep in range(repeats):
                  # stream in all e-group slices upfront (SP HWDGE ring);
                  # record the last load instruction per group for pacing deps
                  xts_g = []
                  gstart = [sum(EGROUPS[:g]) for g in range(G)]
                  for g in range(G):
                      eg = EGROUPS[g] * PT
                      e0 = gstart[g] * PT
                      xts = []
                      for sc in range(NSC):
                          xt_sb = xtpool.tile([PT, eg], F32, tag=f"xt{g}", name=f"xt{g}_{sc}")
                          nc.sync.dma_start(
                              xt_sb[:, :],
                              xt_d[sc * PT:(sc + 1) * PT, e0:e0 + eg],
                          )
                          xts.append(xt_sb)
                      xts_g.append(xts)

                  for g in range(G):
                      xts = xts_g[g]
                      for ii in range(EGROUPS[g]):
                          i = gstart[g] + ii               # global e-tile
                          esl = slice(ii * PT, (ii + 1) * PT)

                          # carries: psum_cy[e, c'] = sum_{s < c'*C} x[e,s]*v[s]*d^..
                          ps_cy = pscy.tile([PT, NCH], F32, tag="cy")
                          for sc in range(NSC - 1):   # R[NSC-1] is all zero
                              nc.tensor.matmul(
                                  ps_cy[:, :],
                                  xts[sc][:, esl],
                                  rmat[:, sc * NCH:(sc + 1) * NCH],
                                  start=(sc == 0), stop=(sc == NSC - 2),
                              )
                          cy_sb = spool.tile([PT, NCH], F32, tag="cys")
                          nc.scalar.copy(cy_sb[:, :], ps_cy[:, :])

                          # within-chunk mains (one matmul per chunk)
                          ps_m = [psm.tile([PT, 4 * C], F32, tag="m", name=f"ps_m{q}")
                                  for q in range(NCH // 4)]
                          for c in range(NCH):
                              dst = ps_m[c // 4][:, (c % 4) * C:(c % 4 + 1) * C]
                              nc.tensor.matmul(
                                  dst,
                                  xts[c][:, esl],
                                  tmat[:, c * C:(c + 1) * C],
                                  start=True, stop=d_is_one,
                              )

                          out_sb = opool.tile([PT, S], F32, tag="o")

                          if d_is_one:
                              # fused copy-out + per-partition carry bias,
                              # split across ScalarE / VectorE
                              for c in range(NCH):
                                  src = ps_m[c // 4][:, (c % 4) * C:(c % 4 + 1) * C]
                                  dstc = out_sb[:, c * C:(c + 1) * C]
                                  if c % 2 == 0:
                                      nc.scalar.add(dstc, src, cy_sb[:, c:c + 1])
                                  else:
                                      nc.vector.tensor_scalar_add(dstc, src, cy_sb[:, c:c + 1])
                          else:
                              # carry * d^{t_l} via rank-1 matmul into the main psum
                              ps_cyT = pscy.tile([NCH, PT], F32, tag="cyT")
                              nc.tensor.transpose(ps_cyT[:, :], cy_sb[:, :], ident[:, :])
                              cyT_sb = spool.tile([NCH, PT], F32, tag="cyTs")
                              nc.scalar.copy(cyT_sb[:, :], ps_cyT[:, :])
                              for c in range(NCH):
                                  dst = ps_m[c // 4][:, (c % 4) * C:(c % 4 + 1) * C]
                                  nc.tensor.matmul(
                                      dst,
                                      cyT_sb[c:c + 1, :],
                                      dpow[:, :],
                                      start=False, stop=True,
                                  )
                              for c in range(NCH):
                                  src = ps_m[c // 4][:, (c % 4) * C:(c % 4 + 1) * C]
                                  dstc = out_sb[:, c * C:(c + 1) * C]
                                  if c % 2 == 0:
                                      nc.scalar.copy(dstc, src)
                                  else:
                                      nc.vector.tensor_copy(dstc, src)

                          # ship output in two halves on the ACT HWDGE ring
                          # (separate FIFO from the input loads on the SP ring)
                          nc.scalar.dma_start(
                              out_d[i * PT:(i + 1) * PT, 0:S // 2], out_sb[:, 0:S // 2])
                          nc.scalar.dma_start(
                              out_d[i * PT:(i + 1) * PT, S // 2:S], out_sb[:, S // 2:S])

    nc.compile()
    _prog_cache[key] = nc
    return nc


def _make_in_maps(xT, Tm, Rm, dpow):
    tmat = Tm.transpose(1, 0, 2).reshape(PT, NSC * C)
    rmat = Rm.transpose(1, 0, 2).reshape(PT, NSC * NCH)
    ident = np.eye(PT, dtype=np.float32)
    return [{"xt": xT[b], "tmat": tmat, "rmat": rmat, "dpow": dpow, "ident": ident}
            for b in range(N_CORES)]


def kernel(x, weight, diag_weight, bias, decay_value):
    x = np.asarray(x, dtype=np.float32)
    v = np.asarray(weight, dtype=np.float32).reshape(-1)
    v2 = np.asarray(diag_weight, dtype=np.float32).reshape(-1)
    bias = np.asarray(bias, dtype=np.float32).reshape(-1)
    d = float(np.clip(np.asarray(decay_value, dtype=np.float32)[1, 0], 0.9, 1.0))

    xT = np.ascontiguousarray(x.transpose(0, 2, 1))   # (B, S, E) layout change
    Tm, Rm, dpow = _build_constants(v, v2, d)
    nc = _build_program(d_is_one=(d == 1.0))

    in_maps = _make_in_maps(xT, Tm, Rm, dpow)
    res = bass_utils.run_bass_kernel_spmd(nc, in_maps, core_ids=list(range(N_CORES)))
    out = np.stack([res.results[b]["out"] for b in range(N_CORES)], axis=0)
    if np.any(bias):
        out = out + bias[None, None, :]
    return out

